# revision 13
# baseline (speedup 1.0000x reference)
"""Trainium2 SPMD kernel: StyleGAN2-style modulated conv (Conv2dWeightModulate).

Reference math (per batch sample b):
    w0        = weight * RC                       (equalized-lr scale)
    ws        = w0 * style[b][None,:,None,None]   (per-input-channel modulation)
    sigma_inv = rsqrt(sum_{I,K,K} ws^2 + eps)     (per-output-channel demodulation)
    out[b]    = conv2d(x[b], ws * sigma_inv, pad=1)

Because the modulation is a per-input-channel scale and conv is linear, this
factorizes into ops with a SHARED weight across the batch:
    out[b] = sigma_inv[b,:] * conv2d(x[b] * (style[b]*RC), weight)
    sigma_inv[b,o] = rsqrt(RC^2 * sum_{i,t} weight[o,i,t]^2 * style[b,i]^2 + eps)

Sharding: data-parallel over batch: 8 samples -> 8 NeuronCores, weight
replicated (the groups=b conv factorizes exactly across the batch).

On-device per core:
  - x (scaled by style*RC on DVE) sits in SBUF as 4 chunks of [128, 34, 34]
    (zero-padded picture), channel chunk = partition dim.
  - conv = 9 taps x 4 input-channel chunks of accumulated 128x128 @ 128x512
    matmuls (float32r: full-rate fp32 path), PSUM groups = 4 out-chunks x 2
    pixel halves.
  - sigma: ACT squares each weight chunk, PE reduces against style^2 via a
    [128,1] lhsT matmul into a [1,512] PSUM bank; sqrt+reciprocal+transpose.
"""

from contextlib import ExitStack

import numpy as np

import concourse.bass as bass
import concourse.tile as tile
from concourse import bacc, mybir
from concourse.bass_utils import run_bass_kernel_spmd

B = 8
CIN = 512
COUT = 512
KK = 3
H = 32
W = 32
PIX = H * W
NCH = 4  # channel chunks of 128
TAPS = KK * KK
RC = float(1.0 / np.sqrt(CIN * KK * KK))
EPS = 1e-8
F32 = mybir.dt.float32
F32R = mybir.dt.float32r
AF = mybir.ActivationFunctionType

# test.py toggles these; the grading harness just calls kernel().
TRACE = False
LAST_RESULTS = None


def _body(ctx, tc, x_d, st_d, wt_d, out_d):
    nc = tc.nc
    const = ctx.enter_context(tc.tile_pool(name="const", bufs=1))
    wpool = ctx.enter_context(tc.tile_pool(name="wpool", bufs=1))
    xpool = ctx.enter_context(tc.tile_pool(name="xpool", bufs=1))
    sqpool = ctx.enter_context(tc.tile_pool(name="sqpool", bufs=3))
    opool = ctx.enter_context(tc.tile_pool(name="opool", bufs=3))
    psum = ctx.enter_context(
        tc.tile_pool(name="psum", bufs=1, space=bass.MemorySpace.PSUM)
    )
    sigpsum = ctx.enter_context(
        tc.tile_pool(name="sigpsum", bufs=1, space=bass.MemorySpace.PSUM)
    )

    # --- style scales ---
    st = const.tile([128, NCH], F32, tag="st")
    nc.sync.dma_start(st[:], st_d[:])
    st_rc = const.tile([128, NCH], F32, tag="st_rc")
    nc.vector.tensor_scalar_mul(st_rc[:], st[:], RC)
    st2 = const.tile([128, NCH], mybir.dt.bfloat16, tag="st2")
    nc.vector.tensor_mul(st2[:], st[:], st[:])

    # --- padded, style-scaled input picture per channel chunk (fp32r) ---
    xs = []
    for c in range(NCH):
        xc = xpool.tile([128, H + 2, W + 2], F32R, tag=f"xs{c}", name=f"xs{c}")
        nc.vector.memset(xc[:].bitcast(F32), 0.0)
        xst = xpool.tile([128, H, W], F32, tag=f"xst{c}", name=f"xst{c}")
        nc.sync.dma_start(xst[:], x_d[c].rearrange("p (h w) -> p h w", h=H))
        nc.vector.tensor_scalar_mul(
            xc[:, 1 : H + 1, 1 : W + 1],
            xst[:],
            st_rc[:, c : c + 1],
        )
        xs.append(xc)

    # --- weights (lhsT layout [i, tap, o]), fp32->fp32r cast during SWDGE DMA ---
    wt = []
    for c in range(NCH):
        wc = wpool.tile([128, TAPS, COUT], F32R, tag=f"wt{c}", name=f"wt{c}")
        nc.gpsimd.dma_start(wc[:], wt_d[:, c])
        wt.append(wc)

    sig_ps = sigpsum.tile([1, COUT], F32, tag="sig")

    groups = [(oc, h) for h in range(2) for oc in range(NCH)]
    wave_a, wave_b = groups[:4], groups[4:]
    pc = {
        g: psum.tile([128, 512], F32, tag=f"pc{i % 4}", name=f"pc{i}")
        for i, g in enumerate(wave_a)
    }

    def conv_mm(g, c, t, start, stop):
        oc, h = g
        dy, dx = t // 3, t % 3
        h0 = h * 16
        nc.tensor.matmul(
            pc[g][:],
            wt[c][:, t, oc * 128 : (oc + 1) * 128],
            xs[c][:, dy + h0 : dy + h0 + 16, dx : dx + W],
            start=start,
            stop=stop,
        )

    def sig_mm(c, t):
        w2 = sqpool.tile([128, COUT], mybir.dt.bfloat16, tag="w2", name="w2")
        nc.scalar.activation(w2[:], wt[c][:, t], AF.Square)
        nc.tensor.matmul(
            sig_ps[:],
            st2[:, c : c + 1],
            w2[:],
            start=(c == 0 and t == 0),
            stop=(c == NCH - 1 and t == TAPS - 1),
        )

    # --- wave A: 4 psum groups + sigma, c-major so compute starts on chunk 0.
    # For the last chunk the sigma matmuls go first so sigma finalization
    # overlaps the tail of wave A on ACT/DVE and the PE never stalls on it.
    for c in range(NCH):
        first = c == 0
        last = c == NCH - 1
        if last:
            for t in range(TAPS):
                sig_mm(c, t)
        for t in range(TAPS):
            if not last:
                sig_mm(c, t)
            for g in wave_a:
                conv_mm(g, c, t, first and t == 0, last and t == TAPS - 1)

    # --- sigma finalize: sqrt(RC^2*q + eps) -> 1/() -> PE-transpose to [128,4] ---
    eps_b = const.tile([1, 1], F32, tag="eps_b")
    nc.vector.memset(eps_b[:], EPS)
    ones_t = const.tile([1, 1], F32, tag="ones_t")
    nc.vector.memset(ones_t[:], 1.0)
    sig_sq = const.tile([1, COUT], F32, tag="sig_sq")
    nc.scalar.activation(sig_sq[:], sig_ps[:], AF.Sqrt, bias=eps_b[:], scale=RC * RC)
    sig_inv = const.tile([1, COUT], F32, tag="sig_inv")
    nc.vector.reciprocal(sig_inv[:], sig_sq[:])
    sig_tp = sigpsum.tile([128, NCH], F32, tag="sig_tp")
    for oc in range(NCH):
        nc.tensor.transpose(
            sig_tp[:, oc : oc + 1],
            sig_inv[0:1, oc * 128 : (oc + 1) * 128],
            ones_t[:],
        )
    sig_t = const.tile([128, NCH], F32, tag="sig_t")
    nc.vector.tensor_copy(sig_t[:], sig_tp[:])

    def flush(g):
        oc, h = g
        ob = opool.tile([128, 512], F32, tag="ob")
        nc.vector.tensor_scalar_mul(ob[:], pc[g][:], sig_t[:, oc : oc + 1])
        nc.sync.dma_start(out_d[oc, :, h * 512 : (h + 1) * 512], ob[:])

    for g in wave_a:
        flush(g)

    # --- wave B: remaining 4 groups; weights fully resident by now ---
    for g in wave_b:
        pc[g] = psum.tile(
            [128, 512], F32, tag=f"pc{wave_b.index(g) % 4}", name=f"pcb{wave_b.index(g)}"
        )
        k = 0
        for t in range(TAPS):
            for c in range(NCH):
                conv_mm(g, c, t, k == 0, k == TAPS * NCH - 1)
                k += 1
        flush(g)


_CACHE = None


def _get_compiled():
    global _CACHE
    if _CACHE is None:
        nc = bacc.Bacc(
            "TRN2", target_bir_lowering=False, debug=False, num_devices=B
        )
        x_d = nc.dram_tensor("x", [NCH, 128, PIX], F32, kind="ExternalInput").ap()
        st_d = nc.dram_tensor("style", [128, NCH], F32, kind="ExternalInput").ap()
        wt_d = nc.dram_tensor(
            "wt", [128, NCH, TAPS, COUT], F32, kind="ExternalInput"
        ).ap()
        out_d = nc.dram_tensor("out", [NCH, 128, PIX], F32, kind="ExternalOutput").ap()
        with tile.TileContext(nc) as tc, ExitStack() as ctx:
            _body(ctx, tc, x_d, st_d, wt_d, out_d)
        nc.compile()
        _CACHE = nc
    return _CACHE


def kernel(x, style, weight):
    """x: (8,512,32,32) f32, style: (8,512) f32, weight: (512,512,3,3) f32
    -> (8,512,32,32) f32"""
    global LAST_RESULTS
    x = np.ascontiguousarray(np.asarray(x, dtype=np.float32))
    style = np.asarray(style, dtype=np.float32)
    weight = np.asarray(weight, dtype=np.float32)

    # Host-side layout only (no arithmetic): lhsT weight layout
    # wt[i_lo, c, t, o] = weight[o, c*128 + i_lo, t//3, t%3]
    wt = np.ascontiguousarray(
        weight.reshape(COUT, NCH, 128, TAPS).transpose(2, 1, 3, 0)
    )
    in_maps = []
    for b in range(B):
        in_maps.append(
            {
                "x": x[b].reshape(NCH, 128, PIX),
                "style": np.ascontiguousarray(style[b].reshape(NCH, 128).T),
                "wt": wt,
            }
        )

    nc = _get_compiled()
    res = run_bass_kernel_spmd(nc, in_maps, list(range(B)), trace=TRACE)
    LAST_RESULTS = res
    out = np.empty((B, COUT, H, W), dtype=np.float32)
    for b in range(B):
        out[b] = res.results[b]["out"].reshape(COUT, H, W)
    return out


# revision 15
# speedup vs baseline: 1.1407x; 1.1407x over previous
"""Trainium2 SPMD kernel: StyleGAN2-style modulated conv (Conv2dWeightModulate).

Reference math (per batch sample b):
    w0        = weight * RC                       (equalized-lr scale)
    ws        = w0 * style[b][None,:,None,None]   (per-input-channel modulation)
    sigma_inv = rsqrt(sum_{I,K,K} ws^2 + eps)     (per-output-channel demodulation)
    out[b]    = conv2d(x[b], ws * sigma_inv, pad=1)

Because the modulation is a per-input-channel scale and conv is linear, this
factorizes into ops with a SHARED weight across the batch:
    out[b] = sigma_inv[b,:] * conv2d(x[b] * (style[b]*RC), weight)
    sigma_inv[b,o] = rsqrt(RC^2 * sum_{i,t} weight[o,i,t]^2 * style[b,i]^2 + eps)

Sharding: data-parallel over batch: 8 samples -> 8 NeuronCores, weight
replicated (the groups=b conv factorizes exactly across the batch).

On-device per core:
  - x (scaled by style*RC on DVE) sits in SBUF as 4 chunks of [128, 34, 34]
    (zero-padded picture), channel chunk = partition dim.
  - conv = 9 taps x 4 input-channel chunks of accumulated 128x128 @ 128x512
    matmuls (float32r: full-rate fp32 path), PSUM groups = 4 out-chunks x 2
    pixel halves.
  - sigma: ACT squares each weight chunk, PE reduces against style^2 via a
    [128,1] lhsT matmul into a [1,512] PSUM bank; sqrt+reciprocal+transpose.
"""

from contextlib import ExitStack

import numpy as np

import concourse.bass as bass
import concourse.tile as tile
from concourse import bacc, mybir
from concourse.bass_utils import run_bass_kernel_spmd

B = 8
CIN = 512
COUT = 512
KK = 3
H = 32
W = 32
PIX = H * W
NCH = 4  # channel chunks of 128
TAPS = KK * KK
RC = float(1.0 / np.sqrt(CIN * KK * KK))
EPS = 1e-8
F32 = mybir.dt.float32
F32R = mybir.dt.float32r
AF = mybir.ActivationFunctionType

# test.py toggles these; the grading harness just calls kernel().
TRACE = False
LAST_RESULTS = None


def _body(ctx, tc, x_d, st_d, wt_d, out_d):
    nc = tc.nc
    const = ctx.enter_context(tc.tile_pool(name="const", bufs=1))
    wpool = ctx.enter_context(tc.tile_pool(name="wpool", bufs=1))
    xpool = ctx.enter_context(tc.tile_pool(name="xpool", bufs=1))
    sqpool = ctx.enter_context(tc.tile_pool(name="sqpool", bufs=3))
    opool = ctx.enter_context(tc.tile_pool(name="opool", bufs=3))
    psum = ctx.enter_context(
        tc.tile_pool(name="psum", bufs=1, space=bass.MemorySpace.PSUM)
    )
    sigpsum = ctx.enter_context(
        tc.tile_pool(name="sigpsum", bufs=1, space=bass.MemorySpace.PSUM)
    )

    # --- style scales ---
    st = const.tile([128, NCH], F32, tag="st")
    nc.sync.dma_start(st[:], st_d[:])
    st_rc = const.tile([128, NCH], F32, tag="st_rc")
    nc.vector.tensor_scalar_mul(st_rc[:], st[:], RC)
    st2 = const.tile([128, NCH], mybir.dt.bfloat16, tag="st2")
    nc.vector.tensor_mul(st2[:], st[:], st[:])

    # --- padded, style-scaled input picture per channel chunk (fp32r) ---
    xs = []
    for c in range(NCH):
        xc = xpool.tile([128, H + 2, W + 2], F32R, tag=f"xs{c}", name=f"xs{c}")
        nc.vector.memset(xc[:].bitcast(F32), 0.0)
        xst = xpool.tile([128, H, W], F32, tag=f"xst{c}", name=f"xst{c}")
        nc.sync.dma_start(xst[:], x_d[c].rearrange("p (h w) -> p h w", h=H))
        nc.vector.tensor_scalar_mul(
            xc[:, 1 : H + 1, 1 : W + 1],
            xst[:],
            st_rc[:, c : c + 1],
        )
        xs.append(xc)

    # --- weights (lhsT layout [i, tap, o]); DRAM side is declared float32r so
    # plain HWDGE DMA works (PE truncates to fp32r precision internally).
    # First chunk is split fine-grained so the PE can start early.
    wt = []
    for c in range(NCH):
        wc = wpool.tile([128, TAPS, COUT], F32R, tag=f"wt{c}", name=f"wt{c}")
        tap_splits = [(0, 3), (3, 6), (6, 9)] if c == 0 else [(0, 9)]
        for lo, hi in tap_splits:
            nc.scalar.dma_start(wc[:, lo:hi], wt_d[:, c, lo:hi])
        wt.append(wc)

    sig_ps = sigpsum.tile([1, COUT], F32, tag="sig")

    groups = [(oc, h) for h in range(2) for oc in range(NCH)]
    wave_a, wave_b = groups[:4], groups[4:]
    pc = {
        g: psum.tile([128, 512], F32, tag=f"pc{i % 4}", name=f"pc{i}")
        for i, g in enumerate(wave_a)
    }

    def conv_mm(g, c, t, start, stop):
        oc, h = g
        dy, dx = t // 3, t % 3
        h0 = h * 16
        nc.tensor.matmul(
            pc[g][:],
            wt[c][:, t, oc * 128 : (oc + 1) * 128],
            xs[c][:, dy + h0 : dy + h0 + 16, dx : dx + W],
            start=start,
            stop=stop,
        )

    BF16 = mybir.dt.bfloat16

    # Per-chunk sum over taps of squared weights (ACT squares, DVE adds):
    # cuts the PE cost of the sigma reduction from 36 matmuls to 4.
    w2s = {}

    def sig_squares(c):
        parts = []
        for t in range(TAPS):
            w2 = sqpool.tile([128, COUT], BF16, tag=f"w2_{t % 3}", name="w2")
            nc.scalar.activation(w2[:], wt[c][:, t], AF.Square)
            parts.append(w2)
            if t == 1:
                acc = sqpool.tile([128, COUT], BF16, tag=f"w2s{c}", name="w2s")
                nc.vector.tensor_add(acc[:], parts[0][:], parts[1][:])
            elif t > 1:
                nc.vector.tensor_add(acc[:], acc[:], parts[-1][:])
        w2s[c] = acc

    def sig_mm(c):
        nc.tensor.matmul(
            sig_ps[:], st2[:, c : c + 1], w2s[c][:], start=(c == 0), stop=(c == NCH - 1)
        )

    def sig_finalize():
        # sqrt(RC^2*q + eps) -> 1/() -> PE-transpose [1,512] -> [128,4]
        nc.scalar.activation(
            sig_sq[:], sig_ps[:], AF.Sqrt, bias=eps_b[:], scale=RC * RC
        )
        nc.vector.reciprocal(sig_inv[:], sig_sq[:])
        for oc in range(NCH):
            nc.tensor.transpose(
                sig_tp[:, oc : oc + 1],
                sig_inv[0:1, oc * 128 : (oc + 1) * 128],
                ones_t[:],
            )
        nc.vector.tensor_copy(sig_t[:], sig_tp[:])

    eps_b = const.tile([1, 1], F32, tag="eps_b")
    nc.vector.memset(eps_b[:], EPS)
    ones_t = const.tile([1, 1], F32, tag="ones_t")
    nc.vector.memset(ones_t[:], 1.0)
    sig_sq = const.tile([1, COUT], F32, tag="sig_sq")
    sig_inv = const.tile([1, COUT], F32, tag="sig_inv")
    sig_tp = sigpsum.tile([128, NCH], F32, tag="sig_tp")
    sig_t = const.tile([128, NCH], F32, tag="sig_t")

    # --- wave A: 4 psum groups, c-major so compute starts on chunk 0.
    # Chunk c's sigma matmul is emitted one chunk later (c3's mid-c3) so the
    # ACT/DVE square+sum pipeline is always ahead of the PE.
    for c in range(NCH):
        first = c == 0
        last = c == NCH - 1
        sig_squares(c)
        for t in range(TAPS):
            if t == 1 and c > 0:
                sig_mm(c - 1)
            if t == 5 and last:
                sig_mm(c)
                sig_finalize()
            for g in wave_a:
                conv_mm(g, c, t, first and t == 0, last and t == TAPS - 1)

    def flush(g):
        oc, h = g
        ob = opool.tile([128, 512], F32, tag="ob")
        nc.vector.tensor_scalar_mul(ob[:], pc[g][:], sig_t[:, oc : oc + 1])
        nc.sync.dma_start(out_d[oc, :, h * 512 : (h + 1) * 512], ob[:])

    for g in wave_a:
        flush(g)

    # --- wave B: remaining 4 groups; weights fully resident by now ---
    for g in wave_b:
        pc[g] = psum.tile(
            [128, 512], F32, tag=f"pc{wave_b.index(g) % 4}", name=f"pcb{wave_b.index(g)}"
        )
        k = 0
        for t in range(TAPS):
            for c in range(NCH):
                conv_mm(g, c, t, k == 0, k == TAPS * NCH - 1)
                k += 1
        flush(g)


_CACHE = None


def _get_compiled():
    global _CACHE
    if _CACHE is None:
        nc = bacc.Bacc(
            "TRN2", target_bir_lowering=False, debug=False, num_devices=B
        )
        x_d = nc.dram_tensor("x", [NCH, 128, PIX], F32, kind="ExternalInput").ap()
        st_d = nc.dram_tensor("style", [128, NCH], F32, kind="ExternalInput").ap()
        wt_d = nc.dram_tensor(
            "wt", [128, NCH, TAPS, COUT], F32R, kind="ExternalInput"
        ).ap()
        out_d = nc.dram_tensor("out", [NCH, 128, PIX], F32, kind="ExternalOutput").ap()
        with tile.TileContext(nc) as tc, ExitStack() as ctx:
            _body(ctx, tc, x_d, st_d, wt_d, out_d)
        nc.compile()
        _CACHE = nc
    return _CACHE


def kernel(x, style, weight):
    """x: (8,512,32,32) f32, style: (8,512) f32, weight: (512,512,3,3) f32
    -> (8,512,32,32) f32"""
    global LAST_RESULTS
    x = np.ascontiguousarray(np.asarray(x, dtype=np.float32))
    style = np.asarray(style, dtype=np.float32)
    weight = np.asarray(weight, dtype=np.float32)

    # Host-side layout only (no arithmetic): lhsT weight layout
    # wt[i_lo, c, t, o] = weight[o, c*128 + i_lo, t//3, t%3]
    wt = np.ascontiguousarray(
        weight.reshape(COUT, NCH, 128, TAPS).transpose(2, 1, 3, 0)
    )
    in_maps = []
    for b in range(B):
        in_maps.append(
            {
                "x": x[b].reshape(NCH, 128, PIX),
                "style": np.ascontiguousarray(style[b].reshape(NCH, 128).T),
                "wt": wt,
            }
        )

    nc = _get_compiled()
    res = run_bass_kernel_spmd(nc, in_maps, list(range(B)), trace=TRACE)
    LAST_RESULTS = res
    out = np.empty((B, COUT, H, W), dtype=np.float32)
    for b in range(B):
        out[b] = res.results[b]["out"].reshape(COUT, H, W)
    return out
